# revision 63
# baseline (speedup 1.0000x reference)
"""Multi-head attention (B=4, S=2048, E=1024, H=16) on 8 TRN2 NeuronCores.

Sharding: core c handles (batch b = c//2, query S-half = c%2) -> 1024 query
rows per core; all 16 heads stay on-core. K/V projections for the full
sequence of a batch are computed (duplicated) on both cores of that batch,
which avoids any cross-core collective.

v3 structure: a fully software-pipelined dataflow schedule. All state the
attention needs (Q^T, K^T, V-with-ones-column) lives in SBUF in bf16; the
projections are decomposed into ~1.7us work units (8 accumulating matmuls +
one evacuation) that are interleaved into the attention loop as PE filler.
A short prelude computes V s-tile 0, Q chunk 0 and K chunk 0; attention for
head pair 0 then starts ~25us in, and the Activation engine's exp stream
(the scalar bottleneck: 2x 1038ns per 512-query group) runs concurrently
with the remaining projection matmuls instead of after them. An `ensure`
step pops producer units from priority queues right before the consumer
group is emitted, so program order always respects dataflow.

Score matmuls keep keys on partitions and queries on the free dim ([k, q])
so the probability matrix is never transposed; the two heads of a pair are
issued on disjoint PE row groups (K=64 contraction at partition offsets
0/64) back-to-back so the hardware can run them concurrently. The softmax
denominator comes from a ones-column appended to V in the PV matmul,
broadcast across partitions with a K=1 matmul, and normalization happens
after PV (64x cheaper than normalizing probs).

Weights for Q/K are fetched as just-in-time per-chunk slabs (the host ships
them pre-gathered per output chunk) to keep the SBUF footprint down. PSUM
accumulation is fp32 throughout; biases are added in fp32. The mask input
is all-ones per the problem spec, so `where(mask==0, -1e9)` is a no-op and
the mask is not shipped to the device.
"""

import sys

sys.path.insert(0, "/opt/trn_rl_repo")

from contextlib import ExitStack

import numpy as np
import ml_dtypes

import concourse.bass as bass
import concourse.bacc as bacc
import concourse.tile as tile
from concourse import mybir
from concourse.bass_utils import run_bass_kernel_spmd

P = 128
DH = 64

F32 = mybir.dt.float32
F32R = mybir.dt.float32r
BF16 = mybir.dt.bfloat16
NPBF = ml_dtypes.bfloat16


def build_core_program(M=1024, S=2048, E=1024, H=16, QT=512, repeat=1):
    """One core's program: full MHA for M query rows against S keys.

    Inputs (per core, bf16): xqt [E,M], xkt [E,S], xvt [E,S]; wvt/wot [E,E]
    (= W.T, [e_in, e_out]); wqs/wks [E,E] chunk-slab layouts (see
    make_in_maps); biases [E] f32. Output: o [M,E] f32.
    """
    assert E % P == 0 and S % (2 * P) == 0 and M % P == 0 and H % 2 == 0
    assert H * DH == E
    EC = E // P
    MC = M // P
    SC = S // P
    HP = H // 2
    assert HP == EC
    QT = min(QT, M)
    NQT = M // QT
    ST = 512
    NST = S // ST
    SS = ST // P  # 128-rows per s-tile

    nc = bacc.Bacc("TRN2", target_bir_lowering=False, debug=False)

    xqt = nc.dram_tensor("xqt", [E, M], BF16, kind="ExternalInput")
    xkt = nc.dram_tensor("xkt", [E, S], BF16, kind="ExternalInput")
    xvt = nc.dram_tensor("xvt", [E, S], BF16, kind="ExternalInput")
    # wqs/wks: slab layout [c, e, j] = W.T[e, c*128+j] so one chunk's
    # stationary [E, 128] slab is contiguous
    wqs = nc.dram_tensor("wqs", [EC * E, P], BF16, kind="ExternalInput")
    wks = nc.dram_tensor("wks", [EC * E, P], BF16, kind="ExternalInput")
    wvt = nc.dram_tensor("wvt", [E, E], BF16, kind="ExternalInput")
    wot = nc.dram_tensor("wot", [E, E], BF16, kind="ExternalInput")
    bq = nc.dram_tensor("bq", [E], F32, kind="ExternalInput")
    bk = nc.dram_tensor("bk", [E], F32, kind="ExternalInput")
    bv = nc.dram_tensor("bv", [E], BF16, kind="ExternalInput")
    bo = nc.dram_tensor("bo", [E], BF16, kind="ExternalInput")
    out = nc.dram_tensor("o", [M, E], F32, kind="ExternalOutput")

    scale = 1.0 / np.sqrt(DH)

    with tile.TileContext(nc) as tc, ExitStack() as ctx:
        consts = ctx.enter_context(tc.tile_pool(name="consts", bufs=1))
        persist = ctx.enter_context(tc.tile_pool(name="persist", bufs=1))

        # --- constants ---
        bq_sb = consts.tile([P, EC], F32)
        bk_sb = consts.tile([P, EC], F32)
        nc.gpsimd.dma_start(out=bq_sb, in_=bq.ap().rearrange("(c p) -> p c", p=P))
        nc.gpsimd.dma_start(out=bk_sb, in_=bk.ap().rearrange("(c p) -> p c", p=P))
        bv_bc = consts.tile([P, E], BF16)
        bo_bc = consts.tile([P, E], BF16)
        nc.gpsimd.dma_start(
            out=bv_bc, in_=bass.AP(tensor=bv, offset=0, ap=[[0, P], [1, E]])
        )
        nc.gpsimd.dma_start(
            out=bo_bc, in_=bass.AP(tensor=bo, offset=0, ap=[[0, P], [1, E]])
        )
        ones_f = consts.tile([P, SS * H], F32)
        nc.vector.memset(ones_f, 1.0)
        ones_r = consts.tile([P, DH], F32R)
        nc.vector.tensor_copy(ones_r, ones_f[:, 0:DH])

        # --- persistent SBUF-resident state (bf16) ---
        qhT = [persist.tile([P, M], BF16, tag=f"qhT{c}", name=f"qhT{c}") for c in range(EC)]
        khT = [persist.tile([P, S], BF16, tag=f"khT{c}", name=f"khT{c}") for c in range(EC)]
        concatT = [persist.tile([P, M], BF16, tag=f"ccT{c}", name=f"ccT{c}") for c in range(EC)]
        # V per s-tile with a ones-column per head: view [p, ss, h, 65]
        vh = [persist.tile([P, SS * H * 65], BF16, tag=f"vh{st}", name=f"vh{st}")
              for st in range(NST)]
        vh4 = [v.rearrange("p (s h d) -> p s h d", h=H, d=65) for v in vh]
        for st in range(NST):
            nc.vector.tensor_copy(
                vh4[st][:, :, :, 64], ones_f.rearrange("p (s h) -> p s h", h=H)
            )

        for rep in range(repeat):
            es_pw = ExitStack()  # wo is read by Phase O: outlives es_w
            pw = es_pw.enter_context(tc.tile_pool(name=f"pW{rep}", bufs=1))
            es_w = ExitStack()
            psl = es_w.enter_context(tc.tile_pool(name=f"pSl{rep}", bufs=1))
            pkx = es_w.enter_context(tc.tile_pool(name=f"pKx{rep}", bufs=1))
            pqx = es_w.enter_context(tc.tile_pool(name=f"pQx{rep}", bufs=1))
            pvx = es_w.enter_context(tc.tile_pool(name=f"pVx{rep}", bufs=2))

            # ---- input + weight DMAs (sync queue, startup-critical first)
            xtv = [None] * NST

            def load_xtv(st, interleave=None, q=None):
                t_ = pvx.tile([P, EC * ST], BF16, tag="xTv", name="xTv")
                for e in range(EC):
                    (q or nc.sync).dma_start(
                        out=t_[:, e * ST : (e + 1) * ST],
                        in_=xvt[e * P : (e + 1) * P, st * ST : (st + 1) * ST],
                    )
                    if interleave is not None:
                        interleave(e)
                return t_

            wv = pw.tile([P, EC * E], BF16, tag="w", name="wv")

            def ilv(e):
                nc.sync.dma_start(
                    out=wv[:, e * E : (e + 1) * E],
                    in_=wvt[e * P : (e + 1) * P, :],
                )

            xtv[0] = load_xtv(0, interleave=ilv)
            wv_t = [wv[:, e * E : (e + 1) * E] for e in range(EC)]

            def xtv_slice(st, ss):
                # [128, 128] stationary slice for s-block (st, ss)
                return lambda e: xtv[st][:, e * ST + ss * P : e * ST + (ss + 1) * P]

            def load_slab(src, c, nm):
                sl = psl.tile([P, EC * P], BF16, tag=nm, name=nm)
                nc.sync.dma_start(
                    out=sl.rearrange("p (ec j) -> p ec j", j=P),
                    in_=src.ap()[c * E : (c + 1) * E, :].rearrange(
                        "(ec p) j -> p ec j", p=P
                    ),
                )
                return [sl[:, e * P : (e + 1) * P] for e in range(EC)]

            qsl = load_slab(wqs, 0, "qsl")
            ksl = load_slab(wks, 0, "ksl")
            xtv[1] = load_xtv(1)
            xTq = pqx.tile([P, EC * M], BF16, tag="xTq", name="xTq")
            for e in range(EC):
                nc.sync.dma_start(
                    out=xTq[:, e * M : (e + 1) * M], in_=xqt[e * P : (e + 1) * P, :]
                )
            xTk = pkx.tile([P, EC * S], BF16, tag="xTk", name="xTk")
            for e in range(EC):
                nc.sync.dma_start(
                    out=xTk[:, e * S : (e + 1) * S], in_=xkt[e * P : (e + 1) * P, :]
                )
            xtv[2] = load_xtv(2)
            xtv[3] = load_xtv(3)
            with tc.tile_pool(name=f"pAp{rep}", bufs=3) as pap, \
                 tc.tile_pool(name=f"pAd{rep}", bufs=1) as pad, \
                 tc.tile_pool(name=f"pFil{rep}", bufs=2, space="PSUM") as pfil, \
                 tc.tile_pool(name=f"pAsc{rep}", bufs=1, space="PSUM") as pasc, \
                 tc.tile_pool(name=f"pAat{rep}", bufs=1, space="PSUM") as paat:

                # ---- work units (emitted as two 4-matmul halves so one
                # half slots between a group's PV pairs) ----
                EH = EC // 2

                def _mm_half(ps, stat_of, mov_of, h):
                    for i in range(EH):
                        e = h * EH + i
                        nc.tensor.matmul(
                            ps, stat_of(e), mov_of(e),
                            start=(e == 0), stop=(e == EC - 1),
                        )

                def _halves(stat, mov, evac, nm):
                    # PSUM tile allocated at emission time (h0) so the pool
                    # ring order matches program order
                    cell = {}

                    def h0():
                        cell["ps"] = pfil.tile([P, 512], F32, tag="fil", name=nm)
                        _mm_half(cell["ps"], stat, mov, 0)

                    def h1():
                        _mm_half(cell["ps"], stat, mov, 1)
                        evac(cell["ps"])

                    return [h0, h1]

                def v_halves(st, ss, oh):
                    def evac(ps):
                        nc.vector.tensor_add(
                            vh4[st][:, ss, oh * 8 : (oh + 1) * 8, 0:64],
                            ps.rearrange("p (h d) -> p h d", d=DH),
                            bv_bc[:, oh * 512 : (oh + 1) * 512].rearrange(
                                "p (h d) -> p h d", d=DH
                            ),
                        )

                    return _halves(
                        xtv_slice(st, ss),
                        lambda e: wv_t[e][:, oh * 512 : (oh + 1) * 512],
                        evac, "vps",
                    )

                def q_halves(c, t, sl):
                    def evac(ps):
                        nc.vector.tensor_scalar_add(
                            qhT[c][:, t * QT : (t + 1) * QT], ps, bq_sb[:, c : c + 1]
                        )

                    return _halves(
                        lambda e: sl[e],
                        lambda e: xTq[:, e * M + t * QT : e * M + (t + 1) * QT],
                        evac, "qps",
                    )

                def k_halves(c, st, sl):
                    def evac(ps):
                        nc.vector.tensor_scalar_add(
                            khT[c][:, st * ST : (st + 1) * ST], ps, bk_sb[:, c : c + 1]
                        )

                    return _halves(
                        lambda e: sl[e],
                        lambda e: xTk[:, e * S + st * ST : e * S + (st + 1) * ST],
                        evac, "kps",
                    )

                def run_unit(halves):
                    for h in halves:
                        h()

                # ---- prelude: V s-tile 0, Q chunk 0, K chunk 0 ----
                for ss in range(SS):
                    for oh in range(E // 512):
                        run_unit(v_halves(0, ss, oh))
                for t in range(NQT):
                    run_unit(q_halves(0, t, qsl))
                for st in range(NST):
                    run_unit(k_halves(0, st, ksl))

                # ---- filler queues (of halves) ----
                vq = []           # (st, half_fn) for st >= 1
                for st in range(1, NST):
                    for ss in range(SS):
                        for oh in range(E // 512):
                            for h in v_halves(st, ss, oh):
                                vq.append((st, h))
                vpop = 0
                cq = []           # q/k halves for chunks >= 1

                def pop_fill():
                    nonlocal vpop
                    if vpop < len(vq):
                        vq[vpop][1]()
                        vpop += 1
                        return True
                    if cq:
                        cq.pop(0)()
                        return True
                    return False

                def ensure_v(st_needed):
                    nonlocal vpop
                    while vpop < len(vq) and vq[vpop][0] <= st_needed:
                        vq[vpop][1]()
                        vpop += 1

                # ---- fused attention + filler loop ----
                wo = None
                for p in range(HP):
                    # queue next chunk's q/k units (slab DMA on gpsimd queue
                    # happens inside load_slab -> sync; keep as-is)
                    if p + 1 < HP:
                        qsl_n = load_slab(wqs, p + 1, "qsl")
                        ksl_n = load_slab(wks, p + 1, "ksl")
                        for t in range(NQT):
                            cq.extend(q_halves(p + 1, t, qsl_n))
                        for st in range(NST):
                            cq.extend(k_halves(p + 1, st, ksl_n))
                    if p == 1:
                        # prefetch O weights into the wv slot (wv dead now)
                        wo = pw.tile([P, EC * E], BF16, tag="w", name="wo")
                        for e in range(EC):
                            nc.sync.dma_start(
                                out=wo[:, e * E : (e + 1) * E],
                                in_=wot[e * P : (e + 1) * P, :],
                            )
                    for t in range(NQT):
                        att = [
                            paat.tile([P, QT], F32, tag=f"att{j}", name=f"att{j}")
                            for j in range(2)
                        ]
                        # depth-2 software pipeline: group g+1's scores+exps
                        # are emitted before group g's PVs, so every PV's
                        # probabilities were computed a full group earlier.
                        # The sc-tag WAR (scores g+1 vs exp g) bounds reuse.
                        def emit_scores(g):
                            pr = []
                            for u in range(2):
                                kc = 2 * g + u
                                scu = pasc.tile(
                                    [P, 2 * QT], F32, tag=f"sc{u}", name=f"sc{u}"
                                )
                                for j in range(2):
                                    nc.tensor.matmul(
                                        scu[:, j * QT : (j + 1) * QT],
                                        khT[p][j * DH : (j + 1) * DH, kc * P : (kc + 1) * P],
                                        qhT[p][j * DH : (j + 1) * DH, t * QT : (t + 1) * QT],
                                        start=True,
                                        stop=True,
                                    )
                                pru = pap.tile(
                                    [P, 2 * QT], BF16, tag=f"pr{u}", name=f"pr{u}"
                                )
                                nc.scalar.activation(
                                    pru, scu, mybir.ActivationFunctionType.Exp,
                                    scale=float(scale),
                                )
                                pr.append(pru)
                            return pr

                        def emit_pvs(g, pr):
                            for u in range(2):
                                kc = 2 * g + u
                                st_, ss_ = kc // SS, kc % SS
                                for j in range(2):
                                    nc.tensor.matmul(
                                        att[j][0:65, :],
                                        vh[st_][:, (ss_ * H + 2 * p + j) * 65 : (ss_ * H + 2 * p + j + 1) * 65],
                                        pr[u][:, j * QT : (j + 1) * QT],
                                        start=(kc == 0),
                                        stop=(kc == SC - 1),
                                    )

                        pending = emit_scores(0)
                        for g in range(SC // 2):
                            if g + 1 < SC // 2:
                                nxt = emit_scores(g + 1)
                            else:
                                nxt = None
                            ensure_v((2 * g + 1) // SS)
                            pop_fill()
                            emit_pvs(g, pending)
                            pending = nxt
                        for j in range(2):
                            den = pad.tile(
                                [65, QT], F32R, tag=f"den{j}", name=f"den{j}"
                            )
                            nc.vector.tensor_copy(den[64:65, :], att[j][64:65, :])
                            dbc = pfil.tile([DH, QT], F32, tag="fil", name="dbc")
                            nc.tensor.matmul(
                                dbc, ones_r[64:65, :], den[64:65, :],
                                start=True, stop=True,
                            )
                            rec = pad.tile([DH, QT], F32, tag=f"rec{j}", name=f"rec{j}")
                            nc.vector.reciprocal_approx_fast(rec, dbc)
                            if j == 0:
                                nc.vector.tensor_mul(
                                    concatT[p][0:DH, t * QT : (t + 1) * QT],
                                    att[j][0:DH, :],
                                    rec,
                                )
                            else:
                                tmp1 = pad.tile([DH, QT], BF16, tag="tmp1", name="tmp1")
                                nc.vector.tensor_mul(tmp1, att[j][0:DH, :], rec)
                                nc.sync.dma_start(
                                    out=concatT[p][DH:P, t * QT : (t + 1) * QT],
                                    in_=tmp1,
                                )

                # drain any leftovers (shouldn't be any)
                while pop_fill():
                    pass
                wo_t = [wo[:, e * E : (e + 1) * E] for e in range(EC)]

            es_w.close()

            # ------------- Phase O: output projection ----------------------
            with tc.tile_pool(name=f"pOn{rep}", bufs=4) as pon, \
                 tc.tile_pool(name=f"pOps{rep}", bufs=6, space="PSUM") as pops:
                for mc in range(MC):
                    for nh in range(E // 512):
                        ps = pops.tile([P, 512], F32, tag="ops", name="ops")
                        for c in range(EC):
                            nc.tensor.matmul(
                                ps,
                                concatT[c][:, mc * P : (mc + 1) * P],
                                wo_t[c][:, nh * 512 : (nh + 1) * 512],
                                start=(c == 0),
                                stop=(c == EC - 1),
                            )
                        ob = pon.tile([P, 512], F32, tag="ob", name="ob")
                        nc.vector.tensor_add(
                            ob, ps, bo_bc[:, nh * 512 : (nh + 1) * 512]
                        )
                        nc.sync.dma_start(
                            out=out[mc * P : (mc + 1) * P, nh * 512 : (nh + 1) * 512],
                            in_=ob,
                        )
            es_pw.close()

    nc.compile()
    return nc


_PROGRAM_CACHE = {}


def _get_program(key=(1024, 2048, 1024, 16)):
    if key not in _PROGRAM_CACHE:
        _PROGRAM_CACHE[key] = build_core_program(*key)
    return _PROGRAM_CACHE[key]


_LAST_RESULTS = None


def _slab_layout(W):
    # wqs[c*E + e, j] = W.T[e, c*128 + j]
    wt = np.ascontiguousarray(np.asarray(W).T)  # [e_in, e_out]
    E = wt.shape[0]
    EC = E // 128
    return np.ascontiguousarray(
        wt.reshape(E, EC, 128).transpose(1, 0, 2).reshape(EC * E, 128)
    ).astype(NPBF)


def make_in_maps(q, k, v, Wq, bq, Wk, bk, Wv, bv, Wo, bo, n_cores=8):
    B, S, E = q.shape
    halves = n_cores // B
    MS = S // halves
    shared = {
        "wqs": _slab_layout(Wq),
        "wks": _slab_layout(Wk),
        "wvt": np.ascontiguousarray(np.asarray(Wv).T).astype(NPBF),
        "wot": np.ascontiguousarray(np.asarray(Wo).T).astype(NPBF),
        "bq": np.asarray(bq, dtype=np.float32), "bk": np.asarray(bk, dtype=np.float32),
        "bv": np.asarray(bv, dtype=np.float32).astype(NPBF),
        "bo": np.asarray(bo, dtype=np.float32).astype(NPBF),
    }
    kT = [np.ascontiguousarray(np.asarray(k[b]).T).astype(NPBF) for b in range(B)]
    vT = [np.ascontiguousarray(np.asarray(v[b]).T).astype(NPBF) for b in range(B)]
    in_maps = []
    for c in range(n_cores):
        b, h = divmod(c, halves)
        in_maps.append({
            "xqt": np.ascontiguousarray(np.asarray(q[b, h * MS : (h + 1) * MS, :]).T).astype(NPBF),
            "xkt": kT[b],
            "xvt": vT[b],
            **shared,
        })
    return in_maps


def kernel(q, k, v, mask, Wq, bq, Wk, bk, Wv, bv, Wo, bo, **run_kwargs):
    q = np.asarray(q, dtype=np.float32)
    k = np.asarray(k, dtype=np.float32)
    v = np.asarray(v, dtype=np.float32)
    B, S, E = q.shape
    n_cores = 8
    halves = n_cores // B
    MS = S // halves
    nc = _get_program((MS, S, E, 16))
    in_maps = make_in_maps(q, k, v, Wq, bq, Wk, bk, Wv, bv, Wo, bo, n_cores)
    res = run_bass_kernel_spmd(nc, in_maps, core_ids=list(range(n_cores)), **run_kwargs)
    global _LAST_RESULTS
    _LAST_RESULTS = res
    out = np.empty((B, S, E), dtype=np.float32)
    for c in range(n_cores):
        b, h = divmod(c, halves)
        out[b, h * MS : (h + 1) * MS, :] = res.results[c]["o"]
    return out


# revision 67
# speedup vs baseline: 1.0131x; 1.0131x over previous
"""Multi-head attention (B=4, S=2048, E=1024, H=16) on 8 TRN2 NeuronCores.

Sharding: core c handles (batch b = c//2, query S-half = c%2) -> 1024 query
rows per core; all 16 heads stay on-core. K/V projections for the full
sequence of a batch are computed (duplicated) on both cores of that batch,
which avoids any cross-core collective.

v3 structure: a fully software-pipelined dataflow schedule. All state the
attention needs (Q^T, K^T, V-with-ones-column) lives in SBUF in bf16; the
projections are decomposed into ~1.7us work units (8 accumulating matmuls +
one evacuation) that are interleaved into the attention loop as PE filler.
A short prelude computes V s-tile 0, Q chunk 0 and K chunk 0; attention for
head pair 0 then starts ~25us in, and the Activation engine's exp stream
(the scalar bottleneck: 2x 1038ns per 512-query group) runs concurrently
with the remaining projection matmuls instead of after them. An `ensure`
step pops producer units from priority queues right before the consumer
group is emitted, so program order always respects dataflow.

Score matmuls keep keys on partitions and queries on the free dim ([k, q])
so the probability matrix is never transposed; the two heads of a pair are
issued on disjoint PE row groups (K=64 contraction at partition offsets
0/64) back-to-back so the hardware can run them concurrently. The softmax
denominator comes from a ones-column appended to V in the PV matmul,
broadcast across partitions with a K=1 matmul, and normalization happens
after PV (64x cheaper than normalizing probs).

Weights for Q/K are fetched as just-in-time per-chunk slabs (the host ships
them pre-gathered per output chunk) to keep the SBUF footprint down. PSUM
accumulation is fp32 throughout; biases are added in fp32. The mask input
is all-ones per the problem spec, so `where(mask==0, -1e9)` is a no-op and
the mask is not shipped to the device.
"""

import sys

sys.path.insert(0, "/opt/trn_rl_repo")

from contextlib import ExitStack

import numpy as np
import ml_dtypes

import concourse.bass as bass
import concourse.bacc as bacc
import concourse.tile as tile
from concourse import mybir
from concourse.bass_utils import run_bass_kernel_spmd

P = 128
DH = 64

F32 = mybir.dt.float32
F32R = mybir.dt.float32r
BF16 = mybir.dt.bfloat16
NPBF = ml_dtypes.bfloat16


def build_core_program(M=1024, S=2048, E=1024, H=16, QT=512, repeat=1):
    """One core's program: full MHA for M query rows against S keys.

    Inputs (per core, bf16): xqt [E,M], xkt [E,S], xvt [E,S]; wvt/wot [E,E]
    (= W.T, [e_in, e_out]); wqs/wks [E,E] chunk-slab layouts (see
    make_in_maps); biases [E] f32. Output: o [M,E] f32.
    """
    assert E % P == 0 and S % (2 * P) == 0 and M % P == 0 and H % 2 == 0
    assert H * DH == E
    EC = E // P
    MC = M // P
    SC = S // P
    HP = H // 2
    assert HP == EC
    QT = min(QT, M)
    NQT = M // QT
    ST = 512
    NST = S // ST
    SS = ST // P  # 128-rows per s-tile

    nc = bacc.Bacc("TRN2", target_bir_lowering=False, debug=False)

    xqt = nc.dram_tensor("xqt", [E, M], BF16, kind="ExternalInput")
    xkt = nc.dram_tensor("xkt", [E, S], BF16, kind="ExternalInput")
    xvt = nc.dram_tensor("xvt", [E, S], BF16, kind="ExternalInput")
    # wqs/wks: slab layout [c, e, j] = W.T[e, c*128+j] so one chunk's
    # stationary [E, 128] slab is contiguous
    wqs = nc.dram_tensor("wqs", [EC * E, P], BF16, kind="ExternalInput")
    wks = nc.dram_tensor("wks", [EC * E, P], BF16, kind="ExternalInput")
    wvt = nc.dram_tensor("wvt", [E, E], BF16, kind="ExternalInput")
    wot = nc.dram_tensor("wot", [E, E], BF16, kind="ExternalInput")
    bq = nc.dram_tensor("bq", [E], F32, kind="ExternalInput")
    bk = nc.dram_tensor("bk", [E], F32, kind="ExternalInput")
    bv = nc.dram_tensor("bv", [E], BF16, kind="ExternalInput")
    bo = nc.dram_tensor("bo", [E], BF16, kind="ExternalInput")
    out = nc.dram_tensor("o", [M, E], F32, kind="ExternalOutput")

    scale = 1.0 / np.sqrt(DH)

    with tile.TileContext(nc) as tc, ExitStack() as ctx:
        consts = ctx.enter_context(tc.tile_pool(name="consts", bufs=1))
        persist = ctx.enter_context(tc.tile_pool(name="persist", bufs=1))

        # --- constants ---
        bq_sb = consts.tile([P, EC], F32)
        bk_sb = consts.tile([P, EC], F32)
        nc.gpsimd.dma_start(out=bq_sb, in_=bq.ap().rearrange("(c p) -> p c", p=P))
        nc.gpsimd.dma_start(out=bk_sb, in_=bk.ap().rearrange("(c p) -> p c", p=P))
        bv_bc = consts.tile([P, E], BF16)
        bo_bc = consts.tile([P, E], BF16)
        nc.gpsimd.dma_start(
            out=bv_bc, in_=bass.AP(tensor=bv, offset=0, ap=[[0, P], [1, E]])
        )
        nc.gpsimd.dma_start(
            out=bo_bc, in_=bass.AP(tensor=bo, offset=0, ap=[[0, P], [1, E]])
        )
        ones_f = consts.tile([P, SS * H], F32)
        nc.vector.memset(ones_f, 1.0)
        ones_r = consts.tile([P, DH], F32R)
        nc.vector.tensor_copy(ones_r, ones_f[:, 0:DH])

        # --- persistent SBUF-resident state (bf16) ---
        qhT = [persist.tile([P, M], BF16, tag=f"qhT{c}", name=f"qhT{c}") for c in range(EC)]
        khT = [persist.tile([P, S], BF16, tag=f"khT{c}", name=f"khT{c}") for c in range(EC)]
        concatT = [persist.tile([P, M], BF16, tag=f"ccT{c}", name=f"ccT{c}") for c in range(EC)]
        # V per s-tile with a ones-column per head: view [p, ss, h, 65]
        vh = [persist.tile([P, SS * H * 65], BF16, tag=f"vh{st}", name=f"vh{st}")
              for st in range(NST)]
        vh4 = [v.rearrange("p (s h d) -> p s h d", h=H, d=65) for v in vh]
        for st in range(NST):
            nc.vector.tensor_copy(
                vh4[st][:, :, :, 64], ones_f.rearrange("p (s h) -> p s h", h=H)
            )

        for rep in range(repeat):
            es_pw = ExitStack()  # wo is read by Phase O: outlives es_w
            pw = es_pw.enter_context(tc.tile_pool(name=f"pW{rep}", bufs=1))
            es_w = ExitStack()
            psl = es_w.enter_context(tc.tile_pool(name=f"pSl{rep}", bufs=1))
            pkx = es_w.enter_context(tc.tile_pool(name=f"pKx{rep}", bufs=1))
            pqx = es_w.enter_context(tc.tile_pool(name=f"pQx{rep}", bufs=1))
            pvx = es_w.enter_context(tc.tile_pool(name=f"pVx{rep}", bufs=2))

            # ---- input + weight DMAs (sync queue, startup-critical first)
            xtv = [None] * NST

            def load_xtv(st, interleave=None, q=None):
                t_ = pvx.tile([P, EC * ST], BF16, tag="xTv", name="xTv")
                for e in range(EC):
                    (q or nc.sync).dma_start(
                        out=t_[:, e * ST : (e + 1) * ST],
                        in_=xvt[e * P : (e + 1) * P, st * ST : (st + 1) * ST],
                    )
                    if interleave is not None:
                        interleave(e)
                return t_

            wv = pw.tile([P, EC * E], BF16, tag="w", name="wv")

            def ilv(e):
                nc.sync.dma_start(
                    out=wv[:, e * E : (e + 1) * E],
                    in_=wvt[e * P : (e + 1) * P, :],
                )

            xtv[0] = load_xtv(0, interleave=ilv)
            wv_t = [wv[:, e * E : (e + 1) * E] for e in range(EC)]

            def xtv_slice(st, ss):
                # [128, 128] stationary slice for s-block (st, ss)
                return lambda e: xtv[st][:, e * ST + ss * P : e * ST + (ss + 1) * P]

            def load_slab(src, c, nm):
                sl = psl.tile([P, EC * P], BF16, tag=nm, name=nm)
                nc.sync.dma_start(
                    out=sl.rearrange("p (ec j) -> p ec j", j=P),
                    in_=src.ap()[c * E : (c + 1) * E, :].rearrange(
                        "(ec p) j -> p ec j", p=P
                    ),
                )
                return [sl[:, e * P : (e + 1) * P] for e in range(EC)]

            qsl = load_slab(wqs, 0, "qsl")
            ksl = load_slab(wks, 0, "ksl")
            xtv[1] = load_xtv(1)
            xTq = pqx.tile([P, EC * M], BF16, tag="xTq", name="xTq")
            for e in range(EC):
                nc.sync.dma_start(
                    out=xTq[:, e * M : (e + 1) * M], in_=xqt[e * P : (e + 1) * P, :]
                )
            xTk = pkx.tile([P, EC * S], BF16, tag="xTk", name="xTk")
            for e in range(EC):
                nc.sync.dma_start(
                    out=xTk[:, e * S : (e + 1) * S], in_=xkt[e * P : (e + 1) * P, :]
                )
            xtv[2] = load_xtv(2)
            xtv[3] = load_xtv(3)
            with tc.tile_pool(name=f"pAp{rep}", bufs=3) as pap, \
                 tc.tile_pool(name=f"pAd{rep}", bufs=1) as pad, \
                 tc.tile_pool(name=f"pFil{rep}", bufs=2, space="PSUM") as pfil, \
                 tc.tile_pool(name=f"pAsc{rep}", bufs=1, space="PSUM") as pasc, \
                 tc.tile_pool(name=f"pAat{rep}", bufs=1, space="PSUM") as paat:

                # ---- work units (emitted as two 4-matmul halves so one
                # half slots between a group's PV pairs) ----
                EH = EC // 2

                def _mm_half(ps, stat_of, mov_of, h):
                    for i in range(EH):
                        e = h * EH + i
                        nc.tensor.matmul(
                            ps, stat_of(e), mov_of(e),
                            start=(e == 0), stop=(e == EC - 1),
                        )

                def _halves(stat, mov, evac, nm):
                    # PSUM tile allocated at emission time (h0) so the pool
                    # ring order matches program order
                    cell = {}

                    def h0():
                        cell["ps"] = pfil.tile([P, 512], F32, tag="fil", name=nm)
                        _mm_half(cell["ps"], stat, mov, 0)

                    def h1():
                        _mm_half(cell["ps"], stat, mov, 1)
                        evac(cell["ps"])

                    return [h0, h1]

                def v_halves(st, ss, oh):
                    def evac(ps):
                        nc.vector.tensor_add(
                            vh4[st][:, ss, oh * 8 : (oh + 1) * 8, 0:64],
                            ps.rearrange("p (h d) -> p h d", d=DH),
                            bv_bc[:, oh * 512 : (oh + 1) * 512].rearrange(
                                "p (h d) -> p h d", d=DH
                            ),
                        )

                    return _halves(
                        xtv_slice(st, ss),
                        lambda e: wv_t[e][:, oh * 512 : (oh + 1) * 512],
                        evac, "vps",
                    )

                def q_halves(c, t, sl):
                    def evac(ps):
                        nc.vector.tensor_scalar_add(
                            qhT[c][:, t * QT : (t + 1) * QT], ps, bq_sb[:, c : c + 1]
                        )

                    return _halves(
                        lambda e: sl[e],
                        lambda e: xTq[:, e * M + t * QT : e * M + (t + 1) * QT],
                        evac, "qps",
                    )

                def k_halves(c, st, sl):
                    def evac(ps):
                        nc.vector.tensor_scalar_add(
                            khT[c][:, st * ST : (st + 1) * ST], ps, bk_sb[:, c : c + 1]
                        )

                    return _halves(
                        lambda e: sl[e],
                        lambda e: xTk[:, e * S + st * ST : e * S + (st + 1) * ST],
                        evac, "kps",
                    )

                def run_unit(halves):
                    for h in halves:
                        h()

                # ---- prelude: V s-tile 0, Q chunk 0, K chunk 0 ----
                for ss in range(SS):
                    for oh in range(E // 512):
                        run_unit(v_halves(0, ss, oh))
                for t in range(NQT):
                    run_unit(q_halves(0, t, qsl))
                for st in range(NST):
                    run_unit(k_halves(0, st, ksl))

                # ---- filler queues (of halves) ----
                vq = []           # (st, half_fn) for st >= 1
                for st in range(1, NST):
                    for ss in range(SS):
                        for oh in range(E // 512):
                            for h in v_halves(st, ss, oh):
                                vq.append((st, h))
                vpop = 0
                cq = []           # q/k halves for chunks >= 1

                def pop_fill():
                    nonlocal vpop
                    if vpop < len(vq):
                        vq[vpop][1]()
                        vpop += 1
                        return True
                    if cq:
                        cq.pop(0)()
                        return True
                    return False

                def ensure_v(st_needed):
                    nonlocal vpop
                    while vpop < len(vq) and vq[vpop][0] <= st_needed:
                        vq[vpop][1]()
                        vpop += 1

                # ---- fused attention + filler loop ----
                wo = None
                for p in range(HP):
                    # queue next chunk's q/k units (slab DMA on gpsimd queue
                    # happens inside load_slab -> sync; keep as-is)
                    if p + 1 < HP:
                        qsl_n = load_slab(wqs, p + 1, "qsl")
                        ksl_n = load_slab(wks, p + 1, "ksl")
                        for t in range(NQT):
                            cq.extend(q_halves(p + 1, t, qsl_n))
                        for st in range(NST):
                            cq.extend(k_halves(p + 1, st, ksl_n))
                    if p == 1:
                        # prefetch O weights into the wv slot (wv dead now)
                        wo = pw.tile([P, EC * E], BF16, tag="w", name="wo")
                        for e in range(EC):
                            nc.sync.dma_start(
                                out=wo[:, e * E : (e + 1) * E],
                                in_=wot[e * P : (e + 1) * P, :],
                            )
                    for t in range(NQT):
                        att = [
                            paat.tile([P, QT], F32, tag=f"att{j}", name=f"att{j}")
                            for j in range(2)
                        ]
                        # depth-2 software pipeline: group g+1's scores+exps
                        # are emitted before group g's PVs, so every PV's
                        # probabilities were computed a full group earlier.
                        # The sc-tag WAR (scores g+1 vs exp g) bounds reuse.
                        def emit_scores(g):
                            pr = []
                            for u in range(2):
                                kc = 2 * g + u
                                scu = pasc.tile(
                                    [P, 2 * QT], F32, tag=f"sc{u}", name=f"sc{u}"
                                )
                                for j in range(2):
                                    nc.tensor.matmul(
                                        scu[:, j * QT : (j + 1) * QT],
                                        khT[p][j * DH : (j + 1) * DH, kc * P : (kc + 1) * P],
                                        qhT[p][j * DH : (j + 1) * DH, t * QT : (t + 1) * QT],
                                        start=True,
                                        stop=True,
                                    )
                                pru = pap.tile(
                                    [P, 2 * QT], BF16, tag=f"pr{u}", name=f"pr{u}"
                                )
                                nc.scalar.activation(
                                    pru, scu, mybir.ActivationFunctionType.Exp,
                                    scale=float(scale),
                                )
                                pr.append(pru)
                            return pr

                        def emit_pvs(g, pr):
                            for u in range(2):
                                kc = 2 * g + u
                                st_, ss_ = kc // SS, kc % SS
                                for j in range(2):
                                    nc.tensor.matmul(
                                        att[j][0:65, :],
                                        vh[st_][:, (ss_ * H + 2 * p + j) * 65 : (ss_ * H + 2 * p + j + 1) * 65],
                                        pr[u][:, j * QT : (j + 1) * QT],
                                        start=(kc == 0),
                                        stop=(kc == SC - 1),
                                    )

                        pending = emit_scores(0)
                        for g in range(SC // 2):
                            if g + 1 < SC // 2:
                                nxt = emit_scores(g + 1)
                            else:
                                nxt = None
                            pop_fill()
                            ensure_v((2 * g + 1) // SS)
                            emit_pvs(g, pending)
                            pending = nxt
                        for j in range(2):
                            den = pad.tile(
                                [65, QT], F32R, tag=f"den{j}", name=f"den{j}"
                            )
                            nc.vector.tensor_copy(den[64:65, :], att[j][64:65, :])
                            dbc = pfil.tile([DH, QT], F32, tag="fil", name="dbc")
                            nc.tensor.matmul(
                                dbc, ones_r[64:65, :], den[64:65, :],
                                start=True, stop=True,
                            )
                            rec = pad.tile([DH, QT], F32, tag=f"rec{j}", name=f"rec{j}")
                            nc.vector.reciprocal_approx_fast(rec, dbc)
                            if j == 0:
                                nc.vector.tensor_mul(
                                    concatT[p][0:DH, t * QT : (t + 1) * QT],
                                    att[j][0:DH, :],
                                    rec,
                                )
                            else:
                                tmp1 = pad.tile([DH, QT], BF16, tag="tmp1", name="tmp1")
                                nc.vector.tensor_mul(tmp1, att[j][0:DH, :], rec)
                                nc.sync.dma_start(
                                    out=concatT[p][DH:P, t * QT : (t + 1) * QT],
                                    in_=tmp1,
                                )

                # drain any leftovers (shouldn't be any)
                while pop_fill():
                    pass
                wo_t = [wo[:, e * E : (e + 1) * E] for e in range(EC)]

            es_w.close()

            # ------------- Phase O: output projection ----------------------
            with tc.tile_pool(name=f"pOn{rep}", bufs=4) as pon, \
                 tc.tile_pool(name=f"pOps{rep}", bufs=6, space="PSUM") as pops:
                for mc in range(MC):
                    for nh in range(E // 512):
                        ps = pops.tile([P, 512], F32, tag="ops", name="ops")
                        for c in range(EC):
                            nc.tensor.matmul(
                                ps,
                                concatT[c][:, mc * P : (mc + 1) * P],
                                wo_t[c][:, nh * 512 : (nh + 1) * 512],
                                start=(c == 0),
                                stop=(c == EC - 1),
                            )
                        ob = pon.tile([P, 512], F32, tag="ob", name="ob")
                        nc.vector.tensor_add(
                            ob, ps, bo_bc[:, nh * 512 : (nh + 1) * 512]
                        )
                        nc.sync.dma_start(
                            out=out[mc * P : (mc + 1) * P, nh * 512 : (nh + 1) * 512],
                            in_=ob,
                        )
            es_pw.close()

    nc.compile()
    return nc


_PROGRAM_CACHE = {}


def _get_program(key=(1024, 2048, 1024, 16)):
    if key not in _PROGRAM_CACHE:
        _PROGRAM_CACHE[key] = build_core_program(*key)
    return _PROGRAM_CACHE[key]


_LAST_RESULTS = None


def _slab_layout(W):
    # wqs[c*E + e, j] = W.T[e, c*128 + j]
    wt = np.ascontiguousarray(np.asarray(W).T)  # [e_in, e_out]
    E = wt.shape[0]
    EC = E // 128
    return np.ascontiguousarray(
        wt.reshape(E, EC, 128).transpose(1, 0, 2).reshape(EC * E, 128)
    ).astype(NPBF)


def make_in_maps(q, k, v, Wq, bq, Wk, bk, Wv, bv, Wo, bo, n_cores=8):
    B, S, E = q.shape
    halves = n_cores // B
    MS = S // halves
    shared = {
        "wqs": _slab_layout(Wq),
        "wks": _slab_layout(Wk),
        "wvt": np.ascontiguousarray(np.asarray(Wv).T).astype(NPBF),
        "wot": np.ascontiguousarray(np.asarray(Wo).T).astype(NPBF),
        "bq": np.asarray(bq, dtype=np.float32), "bk": np.asarray(bk, dtype=np.float32),
        "bv": np.asarray(bv, dtype=np.float32).astype(NPBF),
        "bo": np.asarray(bo, dtype=np.float32).astype(NPBF),
    }
    kT = [np.ascontiguousarray(np.asarray(k[b]).T).astype(NPBF) for b in range(B)]
    vT = [np.ascontiguousarray(np.asarray(v[b]).T).astype(NPBF) for b in range(B)]
    in_maps = []
    for c in range(n_cores):
        b, h = divmod(c, halves)
        in_maps.append({
            "xqt": np.ascontiguousarray(np.asarray(q[b, h * MS : (h + 1) * MS, :]).T).astype(NPBF),
            "xkt": kT[b],
            "xvt": vT[b],
            **shared,
        })
    return in_maps


def kernel(q, k, v, mask, Wq, bq, Wk, bk, Wv, bv, Wo, bo, **run_kwargs):
    q = np.asarray(q, dtype=np.float32)
    k = np.asarray(k, dtype=np.float32)
    v = np.asarray(v, dtype=np.float32)
    B, S, E = q.shape
    n_cores = 8
    halves = n_cores // B
    MS = S // halves
    nc = _get_program((MS, S, E, 16))
    in_maps = make_in_maps(q, k, v, Wq, bq, Wk, bk, Wv, bv, Wo, bo, n_cores)
    res = run_bass_kernel_spmd(nc, in_maps, core_ids=list(range(n_cores)), **run_kwargs)
    global _LAST_RESULTS
    _LAST_RESULTS = res
    out = np.empty((B, S, E), dtype=np.float32)
    for c in range(n_cores):
        b, h = divmod(c, halves)
        out[b, h * MS : (h + 1) * MS, :] = res.results[c]["o"]
    return out
